# revision 1
# baseline (speedup 1.0000x reference)
"""Trainium2 Bass kernel for nn_Attention (no-softmax attention block).

Reference computation (per batch):
    q = x @ Wq.T + bq ; k = x @ Wk.T + bk ; v = x @ Wv.T + bv   (H=12 heads, D=64)
    att = (q k^T) / sqrt(D)      (NO softmax)
    y   = att @ v ;  out = y @ Wp.T + bp

Key algebraic optimization: without softmax, (q k^T) v == q (k^T v).
k^T v is a [D, D] matrix per (batch, head), so the [T, T] attention
matrix is never materialized; attention costs ~2*T*D*D instead of
~2*T*T*D per head.

Sharding: data-parallel over batch (8 cores x 2 batches), no collectives.
Compute: bf16 matmuls, fp32 PSUM accumulation; bf16 biases and bf16
output (host upcasts to f32; all well within the error budget).

Per-core device layout (tokens TOK = 2048):
    xT [C, TOK] (channels on partitions)  ->  QT [C, TOK] (x 1/sqrt(D), +bq)
                                          ->  K, V natural [TOK, C] (+bias)
    M[b,h] = Kh^T @ Vh  [D, D]  (PSUM accumulation over token tiles)
    yT [C, TOK] = M^T-matmuls against QT (head pairs packed in the array)
    out [TOK, C] = yT^T @ WpT + bp

Raw-bass engine programs (no Tile): SP does DMA, PE all matmuls, ACT the
transposed-layout PSUM drains (fused scale/bias), DVE the natural-layout
drains (broadcast bias adds); M/yT phases are software-pipelined and the
yT drains alternate ACT/DVE. Explicit semaphores; waits are standalone
instructions so no 64B-struct sync-slot limits apply. Startup: a small
"boot" DMA (wq col-block 0 + x chunk 0) gates the first matmuls while
dummy matmuls warm the PE clock gate; host-side packs (boot, col-major
wq, one bias tensor with pre-broadcast rows) keep every DMA's contiguous
runs >= 512B and the DMA order aligned with consumption.
"""

import numpy as np
from ml_dtypes import bfloat16

B, T, C, H = 16, 1024, 768, 12
D = C // H                 # 64
N_CORES = 8
BP = B // N_CORES          # batches per core
TOK = BP * T               # tokens per core
CT = C // 128              # 6 channel tiles
TT = TOK // 128            # 16 token tiles
HPAIRS = CT                # 6 head pairs (2 heads per 128-channel tile)
QCH = 512
OCH = 384                  # C split into 2x384 output chunks (1 PSUM bank fp32)
SCALE = 1.0 / float(np.sqrt(D))

_CACHE = {}


def _build_nc():
    import concourse.bass as bass
    from concourse import mybir

    bf16 = mybir.dt.bfloat16
    f32 = mybir.dt.float32
    Ident = mybir.ActivationFunctionType.Identity

    nc = bass.Bass()

    # boot pack: wq cols 0:128 + xT chunk 0, partition-major — the minimal
    # prefix that unblocks the first QT group, in one DMA
    boot_d = nc.declare_dram_parameter("boot", [128, CT, 128 + QCH], bf16, isOutput=False)
    xT_d = nc.declare_dram_parameter("xT", [C, TOK], bf16, isOutput=False)
    # wq packed col-block-major [p, co, a, 128]: per-co DMAs get 1536B
    # contiguous runs (256B runs pay a 2x DMA latency penalty)
    wq_d = nc.declare_dram_parameter("wqp", [128, CT, CT, 128], bf16, isOutput=False)
    wk_d = nc.declare_dram_parameter("wkT", [C, C], bf16, isOutput=False)
    wv_d = nc.declare_dram_parameter("wvT", [C, C], bf16, isOutput=False)
    wp_d = nc.declare_dram_parameter("wpT", [C, C], bf16, isOutput=False)
    # all biases in one tensor: [bqs (partition-major) | bk | bv | bp bcast rows]
    bias_d = nc.declare_dram_parameter("biases", [128, CT + 3 * C], bf16, isOutput=False)
    # bf16 output (host upcasts): halves the output DMA traffic/tail
    out_d = nc.declare_dram_parameter("out", [TOK, C], bf16, isOutput=True)

    def bcast(dram_handle):
        ap = dram_handle[:]
        return bass.AP(tensor=ap.tensor, offset=ap.offset, ap=[[0, 128]] + list(ap.ap))

    import contextlib
    stack = contextlib.ExitStack()
    sb = lambda name, shape, dt: stack.enter_context(nc.sbuf_tensor(name, shape, dt))
    ps = lambda name, shape, dt: stack.enter_context(nc.psum_tensor(name, shape, dt))
    sem = lambda name: stack.enter_context(nc.semaphore(name))

    with stack:
        boot_sb = sb("boot_sb", [128, CT, 128 + QCH], bf16)
        wq_sb = sb("wq_sb", [128, CT, CT, 128], bf16)
        xt_sb = sb("xt_sb", [128, CT, TOK], bf16)
        wk_sb = sb("wk_sb", [128, CT, C], bf16)
        wv_sb = sb("wv_sb", [128, CT, C], bf16)
        wp_sb = sb("wp_sb", [128, CT, C], bf16)
        qt_sb = sb("qt_sb", [128, CT, TOK], bf16)
        k_sb = sb("k_sb", [128, TT, C], bf16)
        v_sb = sb("v_sb", [128, TT, C], bf16)
        m_sb = sb("m_sb", [128, BP * HPAIRS, D], bf16)
        yt_sb = sb("yt_sb", [128, CT, TOK], bf16)
        NOT = 4
        ot_sb = [sb(f"ot_sb{i}", [128, C], bf16) for i in range(NOT)]
        bias_sb = sb("bias_sb", [128, CT + 3 * C], bf16)
        bq_sb = bias_sb[:, 0:CT]
        bk_bc = bias_sb[:, CT:CT + C]
        bv_bc = bias_sb[:, CT + C:CT + 2 * C]
        bp_bc = bias_sb[:, CT + 2 * C:CT + 3 * C]

        proj_ps = [ps(f"proj_ps{i}", [128, QCH], f32) for i in range(3)]
        m_ps = [ps(f"m_ps{i}", [128, D], f32) for i in range(2)]
        py_ps = [ps(f"py_ps{i}", [128, QCH], f32) for i in range(3)]

        sem_boot = sem("s_boot")
        sem_wqc = [sem(f"s_wq{i}") for i in range(CT)]
        sem_xch = [sem(f"s_x{i}") for i in range(TOK // QCH)]
        sem_wk, sem_wv, sem_wp, sem_b = (
            sem("s_wk"), sem("s_wv"), sem("s_wp"), sem("s_b"))
        sem_pe, sem_act, sem_dve = sem("s_pe"), sem("s_act"), sem("s_dve")
        sem_out = [sem(f"s_out{i}") for i in range(NOT)]

        # Defensive semaphore zeroing: allocation does not clear sems, and a
        # prior execution of this NEFF leaves them at final values (all waits
        # would pass immediately -> races). Each semaphore is cleared by the
        # engine that increments it, BEFORE that engine's first increment;
        # the barrier then orders clears against every consumer's first wait.
        # SP (and ACT for the biases) additionally issues its input DMAs
        # before joining the barrier — its sems are already cleared, and no
        # consumer can observe them until after the barrier.
        # The boot DMA (all of wq + x chunk 0 -> the whole first QT sweep)
        # is the startup critical path: clear only its semaphore, issue it,
        # then clear the remaining sems while the transfer runs, then
        # barrier. The bias pack rides the scalar engine's separate HW-DGE
        # ring. Everything else is issued post-barrier so the barrier isn't
        # delayed by DMA issue time.
        nc.sync.sem_clear(sem_boot)
        nc.sync.sem_clear(sem_b)
        for s in sem_wqc[1:]:
            nc.sync.sem_clear(s)
        # pre-barrier SP DMA sequence; ring FIFO fixes the device order:
        # boot (first QT group), then wq per column block (later QT groups),
        # then the bias pack (first PSUM drain)
        nc.sync.dma_start(out=boot_sb[:], in_=boot_d[:]).then_inc(sem_boot, 16)
        for s in (*sem_xch, sem_wk, sem_wv, sem_wp, *sem_out):
            nc.sync.sem_clear(s)
        nc.scalar.sem_clear(sem_act)
        nc.tensor.sem_clear(sem_pe)
        nc.vector.sem_clear(sem_dve)
        # PE warm-up (pre-barrier): ~3.4us of sustained matmuls lifts the
        # HAM clock gate 1.2 -> 2.4 GHz while the boot DMA streams. Inputs
        # are garbage SBUF; the scratch PSUM slot's first real use is much
        # later and opens with start=True.
        for _w in range(8):
            nc.tensor.matmul(py_ps[0][:], xt_sb[:, 0, 0:128], xt_sb[:, 1, 0:QCH],
                             start=True, stop=True)

        nc.all_engine_barrier()

        # ---------------- plan ----------------
        ops = {"sp": [], "pe": [], "act": [], "dve": [], "pool": []}
        cnt = {"pe": 0, "act": 0, "dve": 0}
        waited = {k: {} for k in ops}

        def emit(eng_key, fn):
            ops[eng_key].append(fn)

        def wait(eng_key, s, thr):
            if thr <= 0:
                return
            if waited[eng_key].get(s.name, 0) < thr:
                waited[eng_key][s.name] = thr
                emit(eng_key, lambda e, s=s, t=thr: e.wait_ge(s, t))

        ENG_SEM = {"act": sem_act, "dve": sem_dve}

        # ---- remaining input DMAs (post-barrier, overlap the QT phase).
        # Order on the (serial) DMA fabric matches consumption: wq col
        # blocks 1-2, bias pack (first drain deadline is softened by the
        # 4-deep proj PSUM slots), wq 3-5, then x chunks and wk/wv/wp.
        for co in (1, 2):
            emit("sp", lambda e, co=co, s=sem_wqc[co]: e.dma_start(
                out=wq_sb[:, co, :, :], in_=wq_d[:, co, :, :]).then_inc(s, 16))
        emit("sp", lambda e: e.dma_start(
            out=bias_sb[:], in_=bias_d[:]).then_inc(sem_b, 16))
        for co in (3, 4, 5):
            emit("sp", lambda e, co=co, s=sem_wqc[co]: e.dma_start(
                out=wq_sb[:, co, :, :], in_=wq_d[:, co, :, :]).then_inc(s, 16))
        for tch in range(1, TOK // QCH):
            t0 = tch * QCH
            x_ap = xT_d[:, t0:t0 + QCH].rearrange("(a p) t -> p a t", p=128)
            emit("sp", lambda e, x_ap=x_ap, t0=t0, s=sem_xch[tch]: e.dma_start(
                out=xt_sb[:, :, t0:t0 + QCH], in_=x_ap
            ).then_inc(s, 16))
        for w_sb, w_d, s in ((wk_sb, wk_d, sem_wk),
                             (wv_sb, wv_d, sem_wv), (wp_sb, wp_d, sem_wp)):
            w_ap = w_d[:].rearrange("(a p) c -> p a c", p=128)
            emit("sp", lambda e, w_sb=w_sb, w_ap=w_ap, s=s: e.dma_start(
                out=w_sb[:], in_=w_ap
            ).then_inc(s, 16))

        def xt_slice(ci, t0, n):
            """x-transposed slice; chunk 0 lives in the boot pack."""
            if t0 + n <= QCH:
                return boot_sb[:, ci, 128 + t0:128 + t0 + n]
            return xt_sb[:, ci, t0:t0 + n]

        def wq_slice(ci, co):
            if co == 0:
                return boot_sb[:, ci, 0:128]
            return wq_sb[:, co, ci, :]

        def wait_x(eng, tch):
            wait(eng, sem_boot if tch == 0 else sem_xch[tch], 16)

        def wait_wq(eng, co):
            wait(eng, sem_boot if co == 0 else sem_wqc[co], 16)
        for w_sb, w_d, s in ((wk_sb, wk_d, sem_wk),
                             (wv_sb, wv_d, sem_wv), (wp_sb, wp_d, sem_wp)):
            w_ap = w_d[:].rearrange("(a p) c -> p a c", p=128)
            emit("sp", lambda e, w_sb=w_sb, w_ap=w_ap, s=s: e.dma_start(
                out=w_sb[:], in_=w_ap
            ).then_inc(s, 16))

        proj_tenant = [None] * 3     # (eng_key, cnt) of last drain of this psum slot
        m_tenant = [None, None]
        py_tenant = [None, None, None]

        def slot_wait(tenants, slot):
            t = tenants[slot]
            if t is not None:
                wait("pe", ENG_SEM[t[0]], t[1])

        # ---- Phase QT: QT[o, t] = wqT^T-mm, scale+bias fused into ACT drain
        qt_drain = {}
        g = 0
        for tch in range(TOK // QCH):
            for co in range(CT):
                t0 = tch * QCH
                slot = g % 3
                pq = proj_ps[slot]
                wait_wq("pe", co)
                wait_x("pe", tch)
                slot_wait(proj_tenant, slot)
                for ci in range(CT):
                    mm = lambda e, ci=ci, co=co, t0=t0, pq=pq: e.matmul(
                        pq[:], wq_slice(ci, co), xt_slice(ci, t0, QCH),
                        start=(ci == 0), stop=(ci == CT - 1))
                    if ci == CT - 1:
                        emit("pe", lambda e, mm=mm: mm(e).then_inc(sem_pe))
                        cnt["pe"] += 1
                    else:
                        emit("pe", mm)
                pe_thr = cnt["pe"]
                wait("act", sem_b, 16)
                wait("act", sem_pe, pe_thr)
                emit("act", lambda e, co=co, t0=t0, pq=pq: e.activation(
                    out=qt_sb[:, co, t0:t0 + QCH], in_=pq[:], func=Ident,
                    bias=bq_sb[:, co:co + 1], scale=SCALE
                ).then_inc(sem_act))
                cnt["act"] += 1
                qt_drain[(co, tch)] = cnt["act"]
                proj_tenant[slot] = ("act", cnt["act"])
                g += 1

        # ---- Phases K, V: natural layout [tok, ch], broadcast bias on DVE
        def natural_proj(w_sb, w_sem, dst_sb, bias_bc, drain_dict):
            nonlocal g
            for tt in range(TT):
                for och in range(2):
                    o0 = och * OCH
                    slot = g % 3
                    pv = proj_ps[slot]
                    wait("pe", w_sem, 16)
                    slot_wait(proj_tenant, slot)
                    for ci in range(CT):
                        mm = lambda e, ci=ci, tt=tt, o0=o0, pv=pv, w_sb=w_sb: e.matmul(
                            pv[:, 0:OCH], xt_slice(ci, tt * 128, 128),
                            w_sb[:, ci, o0:o0 + OCH],
                            start=(ci == 0), stop=(ci == CT - 1))
                        if ci == CT - 1:
                            emit("pe", lambda e, mm=mm: mm(e).then_inc(sem_pe))
                            cnt["pe"] += 1
                        else:
                            emit("pe", mm)
                    wait("dve", sem_b, 16)
                    wait("dve", sem_pe, cnt["pe"])
                    emit("dve", lambda e, tt=tt, o0=o0, pv=pv, dst_sb=dst_sb, bias_bc=bias_bc:
                         e.tensor_add(dst_sb[:, tt, o0:o0 + OCH], pv[:, 0:OCH],
                                      bias_bc[:, o0:o0 + OCH]).then_inc(sem_dve))
                    cnt["dve"] += 1
                    drain_dict[(tt, och)] = cnt["dve"]
                    proj_tenant[slot] = ("dve", cnt["dve"])
                    g += 1

        k_drain, v_drain = {}, {}
        natural_proj(wk_sb, sem_wk, k_sb, bk_bc, k_drain)
        natural_proj(wv_sb, sem_wv, v_sb, bv_bc, v_drain)

        # ---- Phases M and yT, software-pipelined: PE computes M(i) while
        # yT(i-1)'s PSUM drains, so the small yT groups never stall on the
        # ACT/DVE drain round-trip.
        # M[b,hpair] = Kh^T @ Vh (both heads col-packed into one PSUM tile);
        # yT[d, q] = M^T-mm against QT (row+col packed pairs).
        m_drain = {}
        yt_drain = {}
        gy = 0

        def m_group(b, hp):
            gm = b * HPAIRS + hp
            slot = gm % 2
            pm = m_ps[slot]
            ochn = (hp * 128) // OCH
            ochn2 = (hp * 128 + 127) // OCH
            slot_wait(m_tenant, slot)
            for kt in range(8):
                tt = b * 8 + kt
                c0 = hp * 128
                need = max(k_drain[(tt, ochn)], v_drain[(tt, ochn)],
                           k_drain[(tt, ochn2)], v_drain[(tt, ochn2)])
                wait("pe", sem_dve, need)
                emit("pe", lambda e, tt=tt, c0=c0, pm=pm, kt=kt: e.matmul(
                    pm[0:D, :], k_sb[:, tt, c0:c0 + D], v_sb[:, tt, c0:c0 + D],
                    start=(kt == 0), stop=(kt == 7), tile_position=(0, 0)))
                mm = lambda e, tt=tt, c0=c0, pm=pm, kt=kt: e.matmul(
                    pm[D:2 * D, :], k_sb[:, tt, c0 + D:c0 + 2 * D],
                    v_sb[:, tt, c0 + D:c0 + 2 * D],
                    start=(kt == 0), stop=(kt == 7), tile_position=(0, 64))
                if kt == 7:
                    emit("pe", lambda e, mm=mm: mm(e).then_inc(sem_pe))
                    cnt["pe"] += 1
                else:
                    emit("pe", mm)
            wait("act", sem_pe, cnt["pe"])
            emit("act", lambda e, gm=gm, pm=pm: e.copy(
                m_sb[:, gm, :], pm[:]).then_inc(sem_act))
            cnt["act"] += 1
            m_drain[gm] = cnt["act"]
            m_tenant[slot] = ("act", cnt["act"])

        def yt_group(b, hp):
            nonlocal gy
            gm = b * HPAIRS + hp
            for qch in range(T // QCH):
                q0 = b * T + qch * QCH
                slot = gy % 3
                py = py_ps[slot]
                wait("pe", sem_act, max(m_drain[gm], qt_drain[(hp, b * 2 + qch)]))
                slot_wait(py_tenant, slot)
                emit("pe", lambda e, gm=gm, hp=hp, q0=q0, py=py: e.matmul(
                    py[0:D, :], m_sb[0:D, gm, :], qt_sb[0:D, hp, q0:q0 + QCH],
                    start=True, stop=True, tile_position=(0, 0)))
                mm = lambda e, gm=gm, hp=hp, q0=q0, py=py: e.matmul(
                    py[D:2 * D, :], m_sb[D:2 * D, gm, :],
                    qt_sb[D:2 * D, hp, q0:q0 + QCH],
                    start=True, stop=True, tile_position=(64, 64))
                emit("pe", lambda e, mm=mm: mm(e).then_inc(sem_pe))
                cnt["pe"] += 1
                # alternate yT drains between ACT and DVE
                dkey = "act" if gy % 2 == 0 else "dve"
                wait(dkey, sem_pe, cnt["pe"])
                if dkey == "act":
                    emit("act", lambda e, hp=hp, q0=q0, py=py: e.copy(
                        yt_sb[:, hp, q0:q0 + QCH], py[:]).then_inc(sem_act))
                else:
                    emit("dve", lambda e, hp=hp, q0=q0, py=py: e.tensor_copy(
                        yt_sb[:, hp, q0:q0 + QCH], py[:]).then_inc(sem_dve))
                cnt[dkey] += 1
                yt_drain[(hp, b, qch)] = (dkey, cnt[dkey])
                py_tenant[slot] = (dkey, cnt[dkey])
                gy += 1

        groups = [(b, hp) for b in range(BP) for hp in range(HPAIRS)]
        for i, (b, hp) in enumerate(groups):
            m_group(b, hp)
            if i > 0:
                yt_group(*groups[i - 1])
        yt_group(*groups[-1])

        # ---- Phase Z: out[t, o] = yT^T-mm + bp, DMA out.  One DMA per tile
        # keeps the SP issue rate low; the last tile is split per-chunk so
        # its first half stores while the second half computes.
        slot_dmas = [0] * NOT
        for tt in range(TT):
            b, qch = tt // 8, (tt % 8) // 4
            slot = tt % NOT
            for och in range(2):
                o0 = och * OCH
                pslot = g % 3
                pz = proj_ps[pslot]
                wait("pe", sem_wp, 16)
                for dkey in ("act", "dve"):
                    need = max((i for k, i in
                                (yt_drain[(hp2, b, qch)] for hp2 in range(CT))
                                if k == dkey), default=0)
                    wait("pe", ENG_SEM[dkey], need)
                slot_wait(proj_tenant, pslot)
                for ci in range(CT):
                    mm = lambda e, ci=ci, tt=tt, o0=o0, pz=pz: e.matmul(
                        pz[:, 0:OCH], yt_sb[:, ci, tt * 128:(tt + 1) * 128],
                        wp_sb[:, ci, o0:o0 + OCH],
                        start=(ci == 0), stop=(ci == CT - 1))
                    if ci == CT - 1:
                        emit("pe", lambda e, mm=mm: mm(e).then_inc(sem_pe))
                        cnt["pe"] += 1
                    else:
                        emit("pe", mm)
                wait("dve", sem_pe, cnt["pe"])
                if och == 0 and tt >= NOT:
                    wait("dve", sem_out[slot], 16 * slot_dmas[slot])
                emit("dve", lambda e, slot=slot, o0=o0, pz=pz: e.tensor_add(
                    ot_sb[slot][:, o0:o0 + OCH], pz[:, 0:OCH],
                    bp_bc[:, o0:o0 + OCH]).then_inc(sem_dve))
                cnt["dve"] += 1
                g += 1
                if tt == TT - 1:
                    wait("sp", sem_dve, cnt["dve"])
                    emit("sp", lambda e, tt=tt, slot=slot, o0=o0: e.dma_start(
                        out=out_d[tt * 128:(tt + 1) * 128, o0:o0 + OCH],
                        in_=ot_sb[slot][:, o0:o0 + OCH]
                    ).then_inc(sem_out[slot], 16))
                    slot_dmas[slot] += 1
            if tt < TT - 1:
                wait("sp", sem_dve, cnt["dve"])
                emit("sp", lambda e, tt=tt, slot=slot: e.dma_start(
                    out=out_d[tt * 128:(tt + 1) * 128, :], in_=ot_sb[slot][:]
                ).then_inc(sem_out[slot], 16))
                slot_dmas[slot] += 1

        # drain: make sure all output DMAs completed before kernel end
        for s_i in range(NOT):
            wait("sp", sem_out[s_i], 16 * slot_dmas[s_i])

        # ---------------- emit ----------------
        with nc.Block(no_gpsimd_drain=True) as block:

            @block.sync
            def _(e):
                for fn in ops["sp"]:
                    fn(e)

            @block.tensor
            def _(e):
                for fn in ops["pe"]:
                    fn(e)

            @block.scalar
            def _(e):
                for fn in ops["act"]:
                    fn(e)

            @block.vector
            def _(e):
                for fn in ops["dve"]:
                    fn(e)

            @block.gpsimd
            def _(e):
                for fn in ops["pool"]:
                    fn(e)

    return nc


def _get_nc():
    if "nc" not in _CACHE:
        _CACHE["nc"] = _build_nc()
    return _CACHE["nc"]


def _make_in_maps(x, Wq, bq, Wk, bk, Wv, bv, Wp, bp):
    wqT = np.ascontiguousarray(Wq.T).astype(bfloat16)
    wkT = np.ascontiguousarray(Wk.T).astype(bfloat16)
    wvT = np.ascontiguousarray(Wv.T).astype(bfloat16)
    wpT = np.ascontiguousarray(Wp.T).astype(bfloat16)
    # [128, CT] partition-major layout: partition p, column ci holds bq[ci*128+p]
    bqs = (bq * SCALE).astype(np.float32).reshape(CT, 128).T
    biases = np.empty((128, CT + 3 * C), dtype=bfloat16)
    biases[:, 0:CT] = bqs
    biases[:, CT:CT + C] = np.broadcast_to(bk.astype(bfloat16), (128, C))
    biases[:, CT + C:CT + 2 * C] = np.broadcast_to(bv.astype(bfloat16), (128, C))
    biases[:, CT + 2 * C:CT + 3 * C] = np.broadcast_to(bp.astype(bfloat16), (128, C))
    # boot[p, a, 0:128] = wqT[a*128+p, 0:128]; boot[p, a, 128:] = xT[a*128+p, 0:512]
    wq_part = wqT[:, 0:128].reshape(CT, 128, 128).transpose(1, 0, 2)
    # wqp[p, co, a, j] = wqT[a*128+p, co*128+j]
    wqp = np.ascontiguousarray(
        wqT.reshape(CT, 128, CT, 128).transpose(1, 2, 0, 3)).astype(bfloat16)
    in_maps = []
    for c in range(N_CORES):
        xs = x[c * BP:(c + 1) * BP].reshape(TOK, C)
        xT = np.ascontiguousarray(xs.T).astype(bfloat16)
        boot = np.empty((128, CT, 128 + QCH), dtype=bfloat16)
        boot[:, :, 0:128] = wq_part
        boot[:, :, 128:] = xT[:, 0:QCH].reshape(CT, 128, QCH).transpose(1, 0, 2)
        in_maps.append({
            "boot": boot, "xT": xT, "wqp": wqp, "wkT": wkT, "wvT": wvT,
            "wpT": wpT, "biases": biases,
        })
    return in_maps


def run(trace=False, tmpdir=None, **inputs):
    from concourse.bass_utils import run_bass_kernel_spmd

    inputs = {k: np.asarray(v, dtype=np.float32) for k, v in inputs.items()}
    nc = _get_nc()
    in_maps = _make_in_maps(**inputs)
    res = run_bass_kernel_spmd(nc, in_maps, core_ids=list(range(N_CORES)),
                               trace=trace, tmpdir=tmpdir)
    out = np.concatenate(
        [res.results[c]["out"].astype(np.float32).reshape(BP, T, C)
         for c in range(N_CORES)], axis=0
    )
    return out, res


def kernel(**inputs):
    out, _ = run(trace=False, **inputs)
    return out



# revision 69
# speedup vs baseline: 1.3453x; 1.3453x over previous
"""Trainium2 Bass kernel for nn_Attention (no-softmax attention block).

Reference computation (per batch):
    q = x @ Wq.T + bq ; k = x @ Wk.T + bk ; v = x @ Wv.T + bv   (H=12 heads, D=64)
    att = (q k^T) / sqrt(D)      (NO softmax)
    y   = att @ v ;  out = y @ Wp.T + bp

Algebra: without softmax, (q k^T) v == q (k^T v), and the output projection
folds through the per-(batch,head) [D,D] matrix:
    out = q @ P + bp,   P[b] rows head-stacked,  P_h = (k^T v)_h^T-free form:
    MT_h = V_h^T K_h  ([j,d]);  P_pair = MT_blockdiag^T-mm against Wp rows.
So the whole block is: 3 projections + tiny MT/P stages + one projection-like
out = q*P. No [T,T] attention matrix, no separate y tensor.

fp8 DoubleRow projections: q/k/v/out-proj matmuls run as fp8e4 DoubleRow
(two 128-deep k-planes per instruction at 0.5 cycles/row). Accuracy is kept
with a 3-term error-corrected split: operands stored as hi + lo fp8 (lo =
quantization residual); product = x_hi*W_hi + x_lo*W_hi + x_hi*W_lo (lo*lo
negligible). Weights pre-scaled by 32 into fp8 range; terms share the scale
so they accumulate raw in PSUM; drains/host fold the scale back. 9 DoubleRow
matmuls replace 6 bf16 ones = 0.75x cycles at ~bf16 accuracy.

Scales (all powers of 2, folded into drains/host): weights fp8 at 32x; qt8
at 32x (psum scale, bias 32bq; the 32 folds into the OUT drain); K/V bf16 at
32x; MT drain x SCALE/1024; P staged x4 (psum -> bf16 -> fp8 hi/lo); OUT
psum = 128*(y WpT), drained x0.25 + 32bp rows = 32*out; host divides by 32.

Drain engine assignment (GPSIMD cannot touch PSUM; TensorScalarPtr is not a
legal GPSIMD opcode — pool gets only plain TensorTensor/copy from SBUF):
  QT:  ACT hi = Q8(psum + 32bq) | DVE lo = (psum + 32bq) - hi (stt).
  K/V: DVE tensor_add (+32*bias rows) -> bf16 at 32x.
  MT:  ACT x2 blockdiag quadrants into zeroed m_sb, scale SCALE/1024.
  P:   ACT stage (x4 -> bf16) + ACT hi fp8 | Pool lo = stage - hi (SBUF).
  OUT: DVE stt (psum*0.25 + 32bp rows) -> bf16 at 32x; host divides by 32.

Schedule: QT (PSUM rotates over all 6 banks) -> K/V(b0) -> one master window
[K/V(b1) och-major + OUT(b0)] with all MT/P units interleaved
proportionally (MT(b1) hp0-2 unblock at the och0 midpoint) -> OUT(b1). No
window is drain-paced and the PE stays gap-free (the cost model's p-state
ramp makes any PE bubble cost ~2x for the next 3us). One serial DMA ring
ordered to stay ahead; PE warm-up matmuls ramp the clock while the boot DMA
(wq col-block 0 + x chunk 0, hi+lo) streams; the last output tile DMAs
per-och so only a 384-wide transfer sits on the tail.
"""

import numpy as np
from ml_dtypes import bfloat16, float8_e4m3

B, T, C, H = 16, 1024, 768, 12
D = C // H                 # 64
N_CORES = 8
BP = B // N_CORES          # batches per core
TOK = BP * T               # tokens per core
CT = C // 128              # 6 channel tiles
CP = CT // 2               # 3 channel-tile pairs (DoubleRow k-planes)
TT = TOK // 128            # 16 token tiles
HPAIRS = CT                # 6 head pairs (2 heads per 128-channel tile)
QCH = 512
OCH = 384                  # C split into 2x384 output chunks (1 PSUM bank fp32)
SCALE = 1.0 / float(np.sqrt(D))
WS = 32.0                  # fp8 weight pre-scale (power of 2)
MS = SCALE / (WS * WS)     # MT drain scale (K and V both carry 32x; fold 1/8)
NOT = 4                    # output staging tiles

# (x term, w term) pairs for the 3-term corrected fp8 product
TERMS = ((0, 0), (1, 0), (0, 1))

_CACHE = {}


def _build_nc():
    import concourse.bass as bass
    from concourse import mybir

    bf16 = mybir.dt.bfloat16
    f32 = mybir.dt.float32
    fp8 = mybir.dt.float8e4
    Ident = mybir.ActivationFunctionType.Identity
    DR = mybir.MatmulPerfMode.DoubleRow
    MULT = mybir.AluOpType.mult
    ADD = mybir.AluOpType.add
    SUB = mybir.AluOpType.subtract

    nc = bass.Bass()

    boot_d = nc.declare_dram_parameter("boot", [128, 2, CT, 128 + QCH], fp8, isOutput=False)
    x8_d = nc.declare_dram_parameter("x8T", [2, C, TOK], fp8, isOutput=False)
    wq8_d = nc.declare_dram_parameter("wq8", [128, CT, 2, CP, 2, 128], fp8, isOutput=False)
    wk8_d = nc.declare_dram_parameter("wk8", [128, 2, CP, 2, C], fp8, isOutput=False)
    wv8_d = nc.declare_dram_parameter("wv8", [128, 2, CP, 2, C], fp8, isOutput=False)
    wp_d = nc.declare_dram_parameter("wpT", [C, C], bf16, isOutput=False)
    # biases: bq (partition-major cols, bf16+f32) early; k/v/p rows later
    bqb_d = nc.declare_dram_parameter("bqb", [128, CT], bf16, isOutput=False)
    bqf_d = nc.declare_dram_parameter("bqf", [128, CT], f32, isOutput=False)
    brows_d = nc.declare_dram_parameter("brows", [128, 3 * C], bf16, isOutput=False)
    out_d = nc.declare_dram_parameter("out", [TOK, C], bf16, isOutput=True)

    import contextlib
    stack = contextlib.ExitStack()
    sb = lambda name, shape, dt: stack.enter_context(nc.sbuf_tensor(name, shape, dt))
    ps = lambda name, shape, dt: stack.enter_context(nc.psum_tensor(name, shape, dt))
    sem = lambda name: stack.enter_context(nc.semaphore(name))

    with stack:
        boot_sb = sb("boot_sb", [128, 2, CT, 128 + QCH], fp8)
        x8_sb = sb("x8_sb", [128, 2, CT, TOK], fp8)
        wq8_sb = sb("wq8_sb", [128, CT, 2, CP, 2, 128], fp8)
        wk8_sb = sb("wk8_sb", [128, 2, CP, 2, C], fp8)
        wv8_sb = sb("wv8_sb", [128, 2, CP, 2, C], fp8)
        wp_sb = sb("wp_sb", [128, CT, C], bf16)
        qt8_sb = sb("qt8_sb", [128, 2, CT, TOK], fp8)   # 32x scale
        k_sb = sb("k_sb", [128, TT, C], bf16)
        v_sb = sb("v_sb", [128, TT, C], bf16)
        m_sb = sb("m_sb", [128, BP * HPAIRS, 128], bf16)
        pbf_sb = sb("pbf_sb", [128, 2, C], bf16)        # P staging (2 pairs)
        p8_sb = sb("p8_sb", [128, 2, BP, CT, C], fp8)
        ot_sb = [sb(f"ot_sb{i}", [128, C], bf16) for i in range(NOT)]
        bq_sb = sb("bq_sb", [128, CT], bf16)
        bqf_sb = sb("bqf_sb", [128, CT], f32)
        brows_sb = sb("brows_sb", [128, 3 * C], bf16)
        bk_bc = brows_sb[:, 0:C]
        bv_bc = brows_sb[:, C:2 * C]
        bp_bc = brows_sb[:, 2 * C:3 * C]

        proj_ps = [ps(f"proj_ps{i}", [128, QCH], f32) for i in range(3)]
        m_ps = [ps(f"m_ps{i}", [128, D], f32) for i in range(2)]
        py_ps = [ps(f"py_ps{i}", [128, QCH], f32) for i in range(3)]

        sem_boot = sem("s_boot")
        sem_wqc = [sem(f"s_wq{i}") for i in range(CT)]
        sem_xch = [sem(f"s_x{i}") for i in range(TOK // QCH)]
        sem_wk, sem_wv, sem_wp, sem_b, sem_br = (
            sem("s_wk"), sem("s_wv"), sem("s_wp"), sem("s_b"), sem("s_br"))
        sem_pe, sem_act, sem_dve, sem_pool = (
            sem("s_pe"), sem("s_act"), sem("s_dve"), sem("s_pool"))
        sem_out = [sem(f"s_out{i}") for i in range(NOT)]

        # Defensive sem zeroing: each sem cleared by its incrementing engine
        # BEFORE its first increment; the barrier orders clears against every
        # consumer's first wait. Boot DMA issues pre-barrier so it streams
        # while the remaining clears + PE warm-up run.
        # Keep SP's pre-barrier stream minimal (it otherwise arrives at the
        # barrier last and delays every engine): SP clears only the boot sem
        # and issues the boot DMA; the DMA-completion sems are cleared by the
        # compute engines (the barrier orders clears before any first wait).
        nc.sync.sem_clear(sem_boot)
        nc.sync.dma_start(out=boot_sb[:], in_=boot_d[:]).then_inc(sem_boot, 16)
        nc.scalar.sem_clear(sem_act)
        for s in sem_wqc[1:]:
            nc.scalar.sem_clear(s)
        nc.vector.sem_clear(sem_dve)
        nc.vector.sem_clear(sem_b)
        nc.vector.sem_clear(sem_br)
        for s in sem_xch:
            nc.vector.sem_clear(s)
        nc.gpsimd.sem_clear(sem_pool)
        for s in (sem_wk, sem_wv, sem_wp, *sem_out):
            nc.gpsimd.sem_clear(s)
        nc.tensor.sem_clear(sem_pe)

        nc.all_engine_barrier()

        # ---------------- plan ----------------
        ops = {"sp": [], "pe": [], "act": [], "dve": [], "pool": []}
        cnt = {"pe": 0, "act": 0, "dve": 0, "pool": 0}
        waited = {k: {} for k in ops}

        def emit(eng_key, fn):
            ops[eng_key].append(fn)

        def wait(eng_key, s, thr):
            if thr <= 0:
                return
            if waited[eng_key].get(s.name, 0) < thr:
                waited[eng_key][s.name] = thr
                emit(eng_key, lambda e, s=s, t=thr: e.wait_ge(s, t))

        ENG_SEM = {"act": sem_act, "dve": sem_dve, "pool": sem_pool}

        # PE warm-up (first post-barrier PE ops): lifts the p-state clock
        # while the boot DMA streams. Pool memsets the warm region first (a
        # real memset — a scale-0 activation would leave NaNs on hardware
        # where uninitialized SBUF is not zero); DVE zeroes the rest of m_sb
        # (off-diagonal blocks for the blockdiag MT).
        emit("pool", lambda e: e.memset(m_sb[:, 0:4, :], 0.0).then_inc(sem_pool))
        cnt["pool"] += 1
        emit("dve", lambda e: e.memset(m_sb[:, 4:, :], 0.0).then_inc(sem_dve))
        cnt["dve"] += 1
        wait("pe", sem_pool, 1)
        for _w in range(7):
            emit("pe", lambda e: e.matmul(
                py_ps[0][:], m_sb[:, 0, :], m_sb[:, 0:4, :],
                start=True, stop=True))

        # ---- input DMAs: one serial ring, ordered to stay ahead.
        emit("sp", lambda e: e.dma_start(
            out=wq8_sb[:, 1], in_=wq8_d[:, 1]).then_inc(sem_wqc[1], 16))
        emit("sp", lambda e: e.dma_start(
            out=bq_sb[:], in_=bqb_d[:]).then_inc(sem_b, 16))
        emit("sp", lambda e: e.dma_start(
            out=bqf_sb[:], in_=bqf_d[:]).then_inc(sem_b, 16))
        for co in (2, 3, 4, 5):
            emit("sp", lambda e, co=co, s=sem_wqc[co]: e.dma_start(
                out=wq8_sb[:, co], in_=wq8_d[:, co]).then_inc(s, 16))
        x_dma_chunks = [1, 2, 3]
        t0 = QCH
        x_ap = x8_d[:, :, t0:t0 + QCH].rearrange("t (a p) x -> p t a x", p=128)
        emit("sp", lambda e, x_ap=x_ap, t0=t0: e.dma_start(
            out=x8_sb[:, :, :, t0:t0 + QCH], in_=x_ap).then_inc(sem_xch[1], 16))
        emit("sp", lambda e: e.dma_start(
            out=brows_sb[:], in_=brows_d[:]).then_inc(sem_br, 16))
        for tch in (2, 3):
            t0 = tch * QCH
            x_ap = x8_d[:, :, t0:t0 + QCH].rearrange("t (a p) x -> p t a x", p=128)
            emit("sp", lambda e, x_ap=x_ap, t0=t0, s=sem_xch[tch]: e.dma_start(
                out=x8_sb[:, :, :, t0:t0 + QCH], in_=x_ap
            ).then_inc(s, 16))
        for w_sb_, w_d_, s in ((wk8_sb, wk8_d, sem_wk), (wv8_sb, wv8_d, sem_wv)):
            emit("sp", lambda e, w_sb_=w_sb_, w_d_=w_d_, s=s: e.dma_start(
                out=w_sb_[:], in_=w_d_[:]).then_inc(s, 16))
        wp_ap = wp_d[:].rearrange("(a p) c -> p a c", p=128)
        emit("sp", lambda e, wp_ap=wp_ap: e.dma_start(
            out=wp_sb[:], in_=wp_ap).then_inc(sem_wp, 16))

        def x_slice(tx, cp, t0, n):
            """xT hi/lo slice [128, 2, n]; chunk 0 lives in the boot pack."""
            if t0 + n <= QCH:
                return boot_sb[:, tx, 2 * cp:2 * cp + 2, 128 + t0:128 + t0 + n]
            return x8_sb[:, tx, 2 * cp:2 * cp + 2, t0:t0 + n]

        def wq_slice(co, tw, cp):
            if co == 0:
                return boot_sb[:, tw, 2 * cp:2 * cp + 2, 0:128]
            return wq8_sb[:, co, tw, cp, :, :]

        def wait_x(eng, tch):
            wait(eng, sem_boot if tch == 0 else sem_xch[tch], 16)

        def wait_wq(eng, co):
            wait(eng, sem_boot if co == 0 else sem_wqc[co], 16)

        all_ps = proj_ps + py_ps     # QT rotates over all 6 (py idle then)
        all_tenant = [None] * 6      # list of (eng_key, cnt) per psum slot
        proj_tenant = all_tenant     # K/V/OUT use slots 0-2
        m_tenant = [None, None]
        pbf_tenant = [None, None]

        def slot_wait(eng, tenants, slot):
            t = tenants[slot]
            if t is not None:
                for ek, ecnt in t:
                    wait(eng, ENG_SEM[ek], ecnt)

        state = {"g": 0, "gp": 0}
        qt_drain = {}                # (co, tch) -> pool lo cnt
        k_drain, v_drain = {}, {}
        m_drain = {}                 # gm -> act cnt
        p_drain = {}                 # (b, hp) -> pool lo cnt

        # ---- unit emitters --------------------------------------------
        def qt_group(tch, co):
            t0 = tch * QCH
            slot = state["g"] % 6
            pq = all_ps[slot]
            wait_wq("pe", co)
            wait_x("pe", tch)
            slot_wait("pe", all_tenant, slot)
            idx = 0
            for tx, tw in TERMS:
                for cp in range(CP):
                    mm = lambda e, tx=tx, tw=tw, cp=cp, co=co, t0=t0, pq=pq, i=idx: e.matmul(
                        pq[:], wq_slice(co, tw, cp), x_slice(tx, cp, t0, QCH),
                        start=(i == 0), stop=(i == 8), perf_mode=DR)
                    if idx == 8:
                        emit("pe", lambda e, mm=mm: mm(e).then_inc(sem_pe))
                        cnt["pe"] += 1
                    else:
                        emit("pe", mm)
                    idx += 1
            # qt8 lives at 32x (psum scale): hi = Q8(psum + 32bq) on ACT;
            # lo = (psum + 32bq) - hi on DVE straight from PSUM. The extra
            # 32 folds into the OUT drain scale.
            wait("act", sem_b, 16)
            wait("act", sem_pe, cnt["pe"])
            emit("act", lambda e, co=co, t0=t0, pq=pq: e.activation(
                out=qt8_sb[:, 0, co, t0:t0 + QCH], in_=pq[:], func=Ident,
                bias=bq_sb[:, co:co + 1]
            ).then_inc(sem_act))
            cnt["act"] += 1
            hi_cnt = cnt["act"]
            wait("dve", sem_b, 32)
            wait("dve", sem_pe, cnt["pe"])
            wait("dve", sem_act, hi_cnt)
            emit("dve", lambda e, pq=pq, co=co, t0=t0: e.scalar_tensor_tensor(
                qt8_sb[:, 1, co, t0:t0 + QCH], pq[:], bqf_sb[:, co:co + 1],
                qt8_sb[:, 0, co, t0:t0 + QCH], ADD, SUB).then_inc(sem_dve))
            cnt["dve"] += 1
            qt_drain[(co, tch)] = cnt["dve"]
            all_tenant[slot] = [("dve", cnt["dve"])]
            state["g"] += 1

        def nat_group(tt, och, w_sb_, w_sem, dst_sb, bias_bc, drain_dict,
                      nslots=3):
            o0 = och * OCH
            slot = state["g"] % nslots
            pv = all_ps[slot]
            wait("pe", w_sem, 16)
            wait_x("pe", (tt * 128) // QCH)
            slot_wait("pe", all_tenant, slot)
            idx = 0
            for tx, tw in TERMS:
                for cp in range(CP):
                    mm = lambda e, tx=tx, tw=tw, cp=cp, tt=tt, o0=o0, pv=pv, w_sb_=w_sb_, i=idx: e.matmul(
                        pv[:, 0:OCH], x_slice(tx, cp, tt * 128, 128),
                        w_sb_[:, tw, cp, :, o0:o0 + OCH],
                        start=(i == 0), stop=(i == 8), perf_mode=DR)
                    if idx == 8:
                        emit("pe", lambda e, mm=mm: mm(e).then_inc(sem_pe))
                        cnt["pe"] += 1
                    else:
                        emit("pe", mm)
                    idx += 1
            wait("dve", sem_br, 16)
            wait("dve", sem_pe, cnt["pe"])
            emit("dve", lambda e, tt=tt, o0=o0, pv=pv, dst_sb=dst_sb, bias_bc=bias_bc:
                 e.tensor_add(dst_sb[:, tt, o0:o0 + OCH], pv[:, 0:OCH],
                              bias_bc[:, o0:o0 + OCH]).then_inc(sem_dve))
            cnt["dve"] += 1
            drain_dict[(tt, och)] = cnt["dve"]
            all_tenant[slot] = [("dve", cnt["dve"])]
            state["g"] += 1

        def m_group(b, hp):
            """MT[b,hpair] = Vh^T @ Kh (transposed M: j on partitions)."""
            gm = b * HPAIRS + hp
            slot = gm % 2
            pm = m_ps[slot]
            ochn = (hp * 128) // OCH
            ochn2 = (hp * 128 + 127) // OCH
            slot_wait("pe", m_tenant, slot)
            for kt in range(8):
                tt = b * 8 + kt
                c0 = hp * 128
                need = max(k_drain[(tt, ochn)], v_drain[(tt, ochn)],
                           k_drain[(tt, ochn2)], v_drain[(tt, ochn2)])
                wait("pe", sem_dve, need)
                emit("pe", lambda e, tt=tt, c0=c0, pm=pm, kt=kt: e.matmul(
                    pm[0:D, :], v_sb[:, tt, c0:c0 + D], k_sb[:, tt, c0:c0 + D],
                    start=(kt == 0), stop=(kt == 7), tile_position=(0, 0)))
                mm = lambda e, tt=tt, c0=c0, pm=pm, kt=kt: e.matmul(
                    pm[D:2 * D, :], v_sb[:, tt, c0 + D:c0 + 2 * D],
                    k_sb[:, tt, c0 + D:c0 + 2 * D],
                    start=(kt == 0), stop=(kt == 7), tile_position=(0, 64))
                if kt == 7:
                    emit("pe", lambda e, mm=mm: mm(e).then_inc(sem_pe))
                    cnt["pe"] += 1
                else:
                    emit("pe", mm)
            # drain diagonal blocks into the zeroed blockdiag tile
            wait("act", sem_pe, cnt["pe"])
            emit("act", lambda e, gm=gm, pm=pm: e.activation(
                out=m_sb[0:D, gm, 0:D], in_=pm[0:D, :], func=Ident,
                scale=MS).then_inc(sem_act))
            cnt["act"] += 1
            emit("act", lambda e, gm=gm, pm=pm: e.activation(
                out=m_sb[D:2 * D, gm, D:2 * D], in_=pm[D:2 * D, :], func=Ident,
                scale=MS).then_inc(sem_act))
            cnt["act"] += 1
            m_drain[gm] = cnt["act"]
            m_tenant[slot] = [("act", cnt["act"])]

        def p_group(b, hp):
            """P_pair = MT_blockdiag^T-mm vs Wp rows; hi/lo fp8 via staging.
            Two independent 384-wide halves (one PSUM bank each)."""
            gm = b * HPAIRS + hp
            wait("pe", sem_wp, 16)
            wait("pe", sem_act, m_drain[gm])
            pslot = gm % 2
            slot_wait("act", pbf_tenant, pslot)
            for och in range(2):
                o0 = och * OCH
                slot = 3 + state["gp"] % 3
                pp = all_ps[slot]
                slot_wait("pe", all_tenant, slot)
                emit("pe", lambda e, gm=gm, hp=hp, o0=o0, pp=pp: e.matmul(
                    pp[:, 0:OCH], m_sb[:, gm, :], wp_sb[:, hp, o0:o0 + OCH],
                    start=True, stop=True).then_inc(sem_pe))
                cnt["pe"] += 1
                # ACT: x4 -> bf16 staging, then hi fp8; Pool: lo = staged - hi
                wait("act", sem_pe, cnt["pe"])
                emit("act", lambda e, pp=pp, pslot=pslot, o0=o0: e.activation(
                    out=pbf_sb[:, pslot, o0:o0 + OCH], in_=pp[:, 0:OCH],
                    func=Ident, scale=4.0).then_inc(sem_act))
                cnt["act"] += 1
                all_tenant[slot] = [("act", cnt["act"])]
                emit("act", lambda e, b=b, hp=hp, pslot=pslot, o0=o0: e.copy(
                    p8_sb[:, 0, b, hp, o0:o0 + OCH],
                    pbf_sb[:, pslot, o0:o0 + OCH]).then_inc(sem_act))
                cnt["act"] += 1
                wait("pool", sem_act, cnt["act"])
                emit("pool", lambda e, b=b, hp=hp, pslot=pslot, o0=o0: e.tensor_sub(
                    p8_sb[:, 1, b, hp, o0:o0 + OCH],
                    pbf_sb[:, pslot, o0:o0 + OCH],
                    p8_sb[:, 0, b, hp, o0:o0 + OCH]).then_inc(sem_pool))
                cnt["pool"] += 1
                state["gp"] += 1
            p_drain[(b, hp)] = cnt["pool"]
            pbf_tenant[pslot] = [("pool", cnt["pool"])]

        slot_dmas = [0] * NOT

        def out_group(tt, och):
            b = tt // 8
            slot = tt % NOT
            o0 = och * OCH
            pslot = state["g"] % 3
            pz = proj_ps[pslot]
            wait("pe", sem_pool, max(p_drain[(b, hp2)] for hp2 in range(CT)))
            wait("pe", sem_dve, max(qt_drain[(co, 2 * b + (tt % 8) // 4)]
                                    for co in range(CT)))
            slot_wait("pe", proj_tenant, pslot)
            idx = 0
            for tq, tp in TERMS:
                for cp in range(CP):
                    mm = lambda e, tq=tq, tp=tp, cp=cp, tt=tt, b=b, o0=o0, pz=pz, i=idx: e.matmul(
                        pz[:, 0:OCH],
                        qt8_sb[:, tq, 2 * cp:2 * cp + 2, tt * 128:(tt + 1) * 128],
                        p8_sb[:, tp, b, 2 * cp:2 * cp + 2, o0:o0 + OCH],
                        start=(i == 0), stop=(i == 8), perf_mode=DR)
                    if idx == 8:
                        emit("pe", lambda e, mm=mm: mm(e).then_inc(sem_pe))
                        cnt["pe"] += 1
                    else:
                        emit("pe", mm)
                    idx += 1
            wait("dve", sem_br, 16)
            wait("dve", sem_pe, cnt["pe"])
            if och == 0 and tt >= NOT:
                wait("dve", sem_out[slot], 16 * slot_dmas[slot])
            # psum = 32q * 4*SCALE*M*WpT = 128*(y WpT); ot = 32*out
            last = tt == TT - 1
            emit("dve", lambda e, slot=slot, o0=o0, pz=pz: e.scalar_tensor_tensor(
                ot_sb[slot][:, o0:o0 + OCH], pz[:, 0:OCH], 0.25,
                bp_bc[:, o0:o0 + OCH], MULT, ADD).then_inc(sem_dve))
            cnt["dve"] += 1
            proj_tenant[pslot] = [("dve", cnt["dve"])]
            state["g"] += 1
            if last:
                # per-och DMA: the och0 transfer hides under och1's compute
                wait("sp", sem_dve, cnt["dve"])
                emit("sp", lambda e, tt=tt, slot=slot, o0=o0: e.dma_start(
                    out=out_d[tt * 128:(tt + 1) * 128, o0:o0 + OCH],
                    in_=ot_sb[slot][:, o0:o0 + OCH]
                ).then_inc(sem_out[slot], 16))
                slot_dmas[slot] += 1
            elif och == 1:
                wait("sp", sem_dve, cnt["dve"])
                emit("sp", lambda e, tt=tt, slot=slot: e.dma_start(
                    out=out_d[tt * 128:(tt + 1) * 128, :], in_=ot_sb[slot][:]
                ).then_inc(sem_out[slot], 16))
                slot_dmas[slot] += 1

        # ---- schedule -------------------------------------------------
        def interleave(la, lb, frac=1.0):
            """Merge work lists; lb paced to finish when la is at `frac`."""
            out, ia, ib = [], 0, 0
            while ia < len(la) or ib < len(lb):
                if ib < len(lb) and (ia >= len(la) or
                                     ib * frac * len(la) <= ia * len(lb)):
                    out.append(lb[ib]); ib += 1
                else:
                    out.append(la[ia]); ia += 1
            return out

        units = []
        for tch in range(TOK // QCH):
            for co in range(CT):
                units.append(lambda tch=tch, co=co: qt_group(tch, co))
                if tch == 3 and co == 2:
                    # hoist the K-phase weight wait into the QT stream (the
                    # DMA landed long ago): the PE.SEQ consumes it while the
                    # engine is still busy, instead of exposing it as a gap
                    # at the phase transition.
                    units.append(lambda: wait("pe", sem_wk, 16))
        # K(b0), V(b0)
        for w_sb_, w_sem, dst, bbc, dd in ((wk8_sb, sem_wk, k_sb, bk_bc, k_drain),
                                           (wv8_sb, sem_wv, v_sb, bv_bc, v_drain)):
            for tt in range(8):
                for och in range(2):
                    units.append(lambda tt=tt, och=och, w_sb_=w_sb_, w_sem=w_sem,
                                 dst=dst, bbc=bbc, dd=dd:
                                 nat_group(tt, och, w_sb_, w_sem, dst, bbc, dd,
                                           nslots=6))
        # Master window: K/V(b1) och-major (och0 for all tt first, so MT(b1)
        # pairs hp0-2 unblock halfway through) followed by OUT(b0); ALL 24
        # MT/P units paced across it so their drains spread over ~34us where
        # ACT/DVE/Pool each stay under ~75% busy.
        kv_b1 = []
        for och in range(2):
            for w_sb_, w_sem, dst, bbc, dd in ((wk8_sb, sem_wk, k_sb, bk_bc, k_drain),
                                               (wv8_sb, sem_wv, v_sb, bv_bc, v_drain)):
                for tt in range(8, 16):
                    kv_b1.append(lambda tt=tt, och=och, w_sb_=w_sb_, w_sem=w_sem,
                                 dst=dst, bbc=bbc, dd=dd:
                                 nat_group(tt, och, w_sb_, w_sem, dst, bbc, dd))

        def mp_units(b):
            """MT/P units software-pipelined: P(hp) follows MT(hp+1), hiding
            the MT ACT-drain round trip behind another PE unit."""
            ms = [lambda hp=hp, b=b: m_group(b, hp) for hp in range(HPAIRS)]
            pse = [lambda hp=hp, b=b: p_group(b, hp) for hp in range(HPAIRS)]
            out = [ms[0]]
            for i in range(1, HPAIRS):
                out += [ms[i], pse[i - 1]]
            out.append(pse[HPAIRS - 1])
            return out

        out_b0 = [lambda tt=tt, och=och: out_group(tt, och)
                  for tt in range(8) for och in range(2)]
        mk = lambda b, hp: (lambda: m_group(b, hp))
        pk = lambda b, hp: (lambda: p_group(b, hp))
        # mpA needs at most K/V(b1) och0 (done at 50% of kv_b1): MT/P(b0)
        # fully + MT(b1, hp0-2). mpB needs och1: the rest, paced over OUT(b0).
        mpA = mp_units(0) + [mk(1, 0), mk(1, 1), pk(1, 0), mk(1, 2), pk(1, 1)]
        mpB = [pk(1, 2), mk(1, 3), mk(1, 4), pk(1, 3), mk(1, 5), pk(1, 4), pk(1, 5)]
        units += interleave(kv_b1, mpA, frac=0.95)
        units += interleave(out_b0, mpB, frac=0.75)
        # tail: OUT(b1)
        units += [lambda tt=tt, och=och: out_group(tt, och)
                  for tt in range(8, 16) for och in range(2)]

        for u in units:
            u()

        for s_i in range(NOT):
            wait("sp", sem_out[s_i], 16 * slot_dmas[s_i])

        # ---------------- emit ----------------
        with nc.Block(no_gpsimd_drain=True) as block:

            @block.sync
            def _(e):
                for fn in ops["sp"]:
                    fn(e)

            @block.tensor
            def _(e):
                for fn in ops["pe"]:
                    fn(e)

            @block.scalar
            def _(e):
                for fn in ops["act"]:
                    fn(e)

            @block.vector
            def _(e):
                for fn in ops["dve"]:
                    fn(e)

            @block.gpsimd
            def _(e):
                for fn in ops["pool"]:
                    fn(e)

    return nc


def _get_nc():
    if "nc" not in _CACHE:
        _CACHE["nc"] = _build_nc()
    return _CACHE["nc"]


def _split8(a):
    hi = a.astype(float8_e4m3)
    lo = (a - hi.astype(np.float32)).astype(float8_e4m3)
    return hi, lo


def _pack_w_nat(wT32):
    """[C, C] -> hi/lo packed [128, 2, CP, 2, C]."""
    hi, lo = _split8(wT32)
    def pack(w):
        return w.reshape(CP, 2, 128, C).transpose(2, 0, 1, 3)
    return np.ascontiguousarray(np.stack([pack(hi), pack(lo)], axis=1))


def _make_in_maps(x, Wq, bq, Wk, bk, Wv, bv, Wp, bp):
    wq32 = np.ascontiguousarray(Wq.T).astype(np.float32) * WS
    wk32 = np.ascontiguousarray(Wk.T).astype(np.float32) * WS
    wv32 = np.ascontiguousarray(Wv.T).astype(np.float32) * WS

    wq_hi, wq_lo = _split8(wq32)
    # wq8[p, co, t, cp, i, j] = wq_t[(2cp+i)*128+p, co*128+j]
    def packq(w):
        return w.reshape(CP, 2, 128, CT, 128).transpose(2, 3, 0, 1, 4)
    wq8 = np.ascontiguousarray(np.stack([packq(wq_hi), packq(wq_lo)], axis=2))

    wk8 = _pack_w_nat(wk32)
    wv8 = _pack_w_nat(wv32)
    wpT = np.ascontiguousarray(Wp.T).astype(bfloat16)

    # [128, CT] partition-major at 32x: partition p, col ci = 32*bq[ci*128+p]
    bqs = (bq * WS).astype(np.float32).reshape(CT, 128).T
    bqf = np.ascontiguousarray(bqs, dtype=np.float32)
    bqb = np.ascontiguousarray(bqs.astype(bfloat16))
    brows = np.empty((128, 3 * C), dtype=bfloat16)
    brows[:, 0:C] = np.broadcast_to((bk * WS).astype(bfloat16), (128, C))
    brows[:, C:2 * C] = np.broadcast_to((bv * WS).astype(bfloat16), (128, C))
    brows[:, 2 * C:3 * C] = np.broadcast_to((bp * WS).astype(bfloat16), (128, C))

    # boot[p, t, ci, 0:128] = wq_t[ci*128+p, 0:128]
    wq_part = np.stack(
        [w[:, 0:128].reshape(CT, 128, 128).transpose(1, 0, 2)
         for w in (wq_hi, wq_lo)], axis=1)

    in_maps = []
    for c in range(N_CORES):
        xs = x[c * BP:(c + 1) * BP].reshape(TOK, C)
        xT = np.ascontiguousarray(xs.T).astype(np.float32)
        xhi, xlo = _split8(xT)
        x8T = np.ascontiguousarray(np.stack([xhi, xlo], axis=0))
        boot = np.empty((128, 2, CT, 128 + QCH), dtype=float8_e4m3)
        boot[:, :, :, 0:128] = wq_part
        for t, xv in enumerate((xhi, xlo)):
            boot[:, t, :, 128:] = xv[:, 0:QCH].reshape(CT, 128, QCH).transpose(1, 0, 2)
        in_maps.append({
            "boot": boot, "x8T": x8T, "wq8": wq8, "wk8": wk8, "wv8": wv8,
            "wpT": wpT, "bqb": bqb, "bqf": bqf, "brows": brows,
        })
    return in_maps


def run(trace=False, tmpdir=None, **inputs):
    from concourse.bass_utils import run_bass_kernel_spmd

    inputs = {k: np.asarray(v, dtype=np.float32) for k, v in inputs.items()}
    nc = _get_nc()
    in_maps = _make_in_maps(**inputs)
    res = run_bass_kernel_spmd(nc, in_maps, core_ids=list(range(N_CORES)),
                               trace=trace, tmpdir=tmpdir)
    out = np.concatenate(
        [(res.results[c]["out"].astype(np.float32) / WS).reshape(BP, T, C)
         for c in range(N_CORES)], axis=0
    )
    return out, res


def kernel(**inputs):
    out, _ = run(trace=False, **inputs)
    return out


# revision 73
# speedup vs baseline: 1.3463x; 1.0008x over previous
"""Trainium2 Bass kernel for nn_Attention (no-softmax attention block).

Reference computation (per batch):
    q = x @ Wq.T + bq ; k = x @ Wk.T + bk ; v = x @ Wv.T + bv   (H=12 heads, D=64)
    att = (q k^T) / sqrt(D)      (NO softmax)
    y   = att @ v ;  out = y @ Wp.T + bp

Algebra: without softmax, (q k^T) v == q (k^T v), and the output projection
folds through the per-(batch,head) [D,D] matrix:
    out = q @ P + bp,   P[b] rows head-stacked,  P_h = (k^T v)_h^T-free form:
    MT_h = V_h^T K_h  ([j,d]);  P_pair = MT_blockdiag^T-mm against Wp rows.
So the whole block is: 3 projections + tiny MT/P stages + one projection-like
out = q*P. No [T,T] attention matrix, no separate y tensor.

fp8 DoubleRow projections: q/k/v/out-proj matmuls run as fp8e4 DoubleRow
(two 128-deep k-planes per instruction at 0.5 cycles/row). Accuracy is kept
with a 3-term error-corrected split: operands stored as hi + lo fp8 (lo =
quantization residual); product = x_hi*W_hi + x_lo*W_hi + x_hi*W_lo (lo*lo
negligible). Weights pre-scaled by 32 into fp8 range; terms share the scale
so they accumulate raw in PSUM; drains/host fold the scale back. 9 DoubleRow
matmuls replace 6 bf16 ones = 0.75x cycles at ~bf16 accuracy.

Scales (all powers of 2, folded into drains/host): weights fp8 at 32x; qt8
at 32x (psum scale, bias 32bq; the 32 folds into the OUT drain); K/V bf16 at
32x; MT drain x SCALE/1024; P staged x4 (psum -> bf16 -> fp8 hi/lo); OUT
psum = 128*(y WpT), drained x0.25 + 32bp rows = 32*out; host divides by 32.

Drain engine assignment (GPSIMD cannot touch PSUM; TensorScalarPtr is not a
legal GPSIMD opcode — pool gets only plain TensorTensor/copy from SBUF):
  QT:  ACT hi = Q8(psum + 32bq) | DVE lo = (psum + 32bq) - hi (stt).
  K/V: DVE tensor_add (+32*bias rows) -> bf16 at 32x.
  MT:  ACT x2 blockdiag quadrants into zeroed m_sb, scale SCALE/1024.
  P:   ACT stage (x4 -> bf16) + ACT hi fp8 | Pool lo = stage - hi (SBUF).
  OUT: DVE stt (psum*0.25 + 32bp rows) -> bf16 at 32x; host divides by 32.

Schedule: QT (PSUM rotates over all 6 banks) -> K/V(b0) -> one master window
[K/V(b1) och-major + OUT(b0)] with all MT/P units interleaved
proportionally (MT(b1) hp0-2 unblock at the och0 midpoint) -> OUT(b1). No
window is drain-paced and the PE stays gap-free (the cost model's p-state
ramp makes any PE bubble cost ~2x for the next 3us). One serial DMA ring
ordered to stay ahead; PE warm-up matmuls ramp the clock while the boot DMA
(wq col-block 0 + x chunk 0, hi+lo) streams; the last output tile DMAs
per-och so only a 384-wide transfer sits on the tail.
"""

import numpy as np
from ml_dtypes import bfloat16, float8_e4m3

B, T, C, H = 16, 1024, 768, 12
D = C // H                 # 64
N_CORES = 8
BP = B // N_CORES          # batches per core
TOK = BP * T               # tokens per core
CT = C // 128              # 6 channel tiles
CP = CT // 2               # 3 channel-tile pairs (DoubleRow k-planes)
TT = TOK // 128            # 16 token tiles
HPAIRS = CT                # 6 head pairs (2 heads per 128-channel tile)
QCH = 512
OCH = 384                  # C split into 2x384 output chunks (1 PSUM bank fp32)
SCALE = 1.0 / float(np.sqrt(D))
WS = 32.0                  # fp8 weight pre-scale (power of 2)
MS = SCALE / (WS * WS)     # MT drain scale (K and V both carry 32x; fold 1/8)
NOT = 4                    # output staging tiles

# (x term, w term) pairs for the 3-term corrected fp8 product
TERMS = ((0, 0), (0, 1), (1, 0))

_CACHE = {}


def _build_nc():
    import concourse.bass as bass
    from concourse import mybir

    bf16 = mybir.dt.bfloat16
    f32 = mybir.dt.float32
    fp8 = mybir.dt.float8e4
    Ident = mybir.ActivationFunctionType.Identity
    DR = mybir.MatmulPerfMode.DoubleRow
    MULT = mybir.AluOpType.mult
    ADD = mybir.AluOpType.add
    SUB = mybir.AluOpType.subtract

    nc = bass.Bass()

    boot_d = nc.declare_dram_parameter("boot", [128, CT, 256 + QCH], fp8, isOutput=False)
    boot2_d = nc.declare_dram_parameter("boot2", [128, CT, QCH], fp8, isOutput=False)
    x8_d = nc.declare_dram_parameter("x8T", [2, C, TOK], fp8, isOutput=False)
    wq8_d = nc.declare_dram_parameter("wq8", [128, CT, 2, CP, 2, 128], fp8, isOutput=False)
    wk8_d = nc.declare_dram_parameter("wk8", [128, 2, CP, 2, C], fp8, isOutput=False)
    wv8_d = nc.declare_dram_parameter("wv8", [128, 2, CP, 2, C], fp8, isOutput=False)
    wp_d = nc.declare_dram_parameter("wpT", [C, C], bf16, isOutput=False)
    # biases: bq (partition-major cols, bf16+f32) early; k/v/p rows later
    bqb_d = nc.declare_dram_parameter("bqb", [128, CT], bf16, isOutput=False)
    bqf_d = nc.declare_dram_parameter("bqf", [128, CT], f32, isOutput=False)
    brows_d = nc.declare_dram_parameter("brows", [128, 3 * C], bf16, isOutput=False)
    out_d = nc.declare_dram_parameter("out", [TOK, C], bf16, isOutput=True)

    import contextlib
    stack = contextlib.ExitStack()
    sb = lambda name, shape, dt: stack.enter_context(nc.sbuf_tensor(name, shape, dt))
    ps = lambda name, shape, dt: stack.enter_context(nc.psum_tensor(name, shape, dt))
    sem = lambda name: stack.enter_context(nc.semaphore(name))

    with stack:
        boot_sb = sb("boot_sb", [128, CT, 256 + QCH], fp8)
        boot2_sb = sb("boot2_sb", [128, CT, QCH], fp8)
        x8_sb = sb("x8_sb", [128, 2, CT, TOK], fp8)
        wq8_sb = sb("wq8_sb", [128, CT, 2, CP, 2, 128], fp8)
        wk8_sb = sb("wk8_sb", [128, 2, CP, 2, C], fp8)
        wv8_sb = sb("wv8_sb", [128, 2, CP, 2, C], fp8)
        wp_sb = sb("wp_sb", [128, CT, C], bf16)
        qt8_sb = sb("qt8_sb", [128, 2, CT, TOK], fp8)   # 32x scale
        k_sb = sb("k_sb", [128, TT, C], bf16)
        v_sb = sb("v_sb", [128, TT, C], bf16)
        m_sb = sb("m_sb", [128, BP * HPAIRS, 128], bf16)
        pbf_sb = sb("pbf_sb", [128, 2, C], bf16)        # P staging (2 pairs)
        p8_sb = sb("p8_sb", [128, 2, BP, CT, C], fp8)
        ot_sb = [sb(f"ot_sb{i}", [128, C], bf16) for i in range(NOT)]
        bq_sb = sb("bq_sb", [128, CT], bf16)
        bqf_sb = sb("bqf_sb", [128, CT], f32)
        brows_sb = sb("brows_sb", [128, 3 * C], bf16)
        bk_bc = brows_sb[:, 0:C]
        bv_bc = brows_sb[:, C:2 * C]
        bp_bc = brows_sb[:, 2 * C:3 * C]

        proj_ps = [ps(f"proj_ps{i}", [128, QCH], f32) for i in range(3)]
        m_ps = [ps(f"m_ps{i}", [128, D], f32) for i in range(2)]
        py_ps = [ps(f"py_ps{i}", [128, QCH], f32) for i in range(3)]

        sem_boot = sem("s_boot")
        sem_boot2 = sem("s_boot2")
        sem_wqc = [sem(f"s_wq{i}") for i in range(CT)]
        sem_xch = [sem(f"s_x{i}") for i in range(TOK // QCH)]
        sem_wk, sem_wv, sem_wp, sem_b, sem_br = (
            sem("s_wk"), sem("s_wv"), sem("s_wp"), sem("s_b"), sem("s_br"))
        sem_pe, sem_act, sem_dve, sem_pool = (
            sem("s_pe"), sem("s_act"), sem("s_dve"), sem("s_pool"))
        sem_out = [sem(f"s_out{i}") for i in range(NOT)]

        # Defensive sem zeroing: each sem cleared by its incrementing engine
        # BEFORE its first increment; the barrier orders clears against every
        # consumer's first wait. Boot DMA issues pre-barrier so it streams
        # while the remaining clears + PE warm-up run.
        # Keep SP's pre-barrier stream minimal (it otherwise arrives at the
        # barrier last and delays every engine): SP clears only the boot sem
        # and issues the boot DMA; the DMA-completion sems are cleared by the
        # compute engines (the barrier orders clears before any first wait).
        nc.sync.sem_clear(sem_boot)
        nc.sync.sem_clear(sem_boot2)
        nc.sync.dma_start(out=boot_sb[:], in_=boot_d[:]).then_inc(sem_boot, 16)
        nc.scalar.sem_clear(sem_act)
        for s in sem_wqc[1:]:
            nc.scalar.sem_clear(s)
        nc.vector.sem_clear(sem_dve)
        nc.vector.sem_clear(sem_b)
        nc.vector.sem_clear(sem_br)
        for s in sem_xch:
            nc.vector.sem_clear(s)
        nc.gpsimd.sem_clear(sem_pool)
        for s in (sem_wk, sem_wv, sem_wp, *sem_out):
            nc.gpsimd.sem_clear(s)
        nc.tensor.sem_clear(sem_pe)

        nc.all_engine_barrier()

        # ---------------- plan ----------------
        ops = {"sp": [], "pe": [], "act": [], "dve": [], "pool": []}
        cnt = {"pe": 0, "act": 0, "dve": 0, "pool": 0}
        waited = {k: {} for k in ops}

        def emit(eng_key, fn):
            ops[eng_key].append(fn)

        def wait(eng_key, s, thr):
            if thr <= 0:
                return
            if waited[eng_key].get(s.name, 0) < thr:
                waited[eng_key][s.name] = thr
                emit(eng_key, lambda e, s=s, t=thr: e.wait_ge(s, t))

        ENG_SEM = {"act": sem_act, "dve": sem_dve, "pool": sem_pool}

        # PE warm-up (first post-barrier PE ops): lifts the p-state clock
        # while the boot DMA streams. Pool memsets the warm region first (a
        # real memset — a scale-0 activation would leave NaNs on hardware
        # where uninitialized SBUF is not zero); DVE zeroes the rest of m_sb
        # (off-diagonal blocks for the blockdiag MT).
        emit("pool", lambda e: e.memset(m_sb[:, 0:4, :], 0.0).then_inc(sem_pool))
        cnt["pool"] += 1
        emit("dve", lambda e: e.memset(m_sb[:, 4:, :], 0.0).then_inc(sem_dve))
        cnt["dve"] += 1
        wait("pe", sem_pool, 1)
        for _w in range(7):
            emit("pe", lambda e: e.matmul(
                py_ps[0][:], m_sb[:, 0, :], m_sb[:, 0:4, :],
                start=True, stop=True))

        # ---- input DMAs: one serial ring, ordered to stay ahead.
        emit("sp", lambda e: e.dma_start(
            out=boot2_sb[:], in_=boot2_d[:]).then_inc(sem_boot2, 16))
        emit("sp", lambda e: e.dma_start(
            out=wq8_sb[:, 1], in_=wq8_d[:, 1]).then_inc(sem_wqc[1], 16))
        emit("sp", lambda e: e.dma_start(
            out=bq_sb[:], in_=bqb_d[:]).then_inc(sem_b, 16))
        emit("sp", lambda e: e.dma_start(
            out=bqf_sb[:], in_=bqf_d[:]).then_inc(sem_b, 16))
        for co in (2, 3, 4, 5):
            emit("sp", lambda e, co=co, s=sem_wqc[co]: e.dma_start(
                out=wq8_sb[:, co], in_=wq8_d[:, co]).then_inc(s, 16))
        x_dma_chunks = [1, 2, 3]
        t0 = QCH
        x_ap = x8_d[:, :, t0:t0 + QCH].rearrange("t (a p) x -> p t a x", p=128)
        emit("sp", lambda e, x_ap=x_ap, t0=t0: e.dma_start(
            out=x8_sb[:, :, :, t0:t0 + QCH], in_=x_ap).then_inc(sem_xch[1], 16))
        emit("sp", lambda e: e.dma_start(
            out=brows_sb[:], in_=brows_d[:]).then_inc(sem_br, 16))
        for tch in (2, 3):
            t0 = tch * QCH
            x_ap = x8_d[:, :, t0:t0 + QCH].rearrange("t (a p) x -> p t a x", p=128)
            emit("sp", lambda e, x_ap=x_ap, t0=t0, s=sem_xch[tch]: e.dma_start(
                out=x8_sb[:, :, :, t0:t0 + QCH], in_=x_ap
            ).then_inc(s, 16))
        for w_sb_, w_d_, s in ((wk8_sb, wk8_d, sem_wk), (wv8_sb, wv8_d, sem_wv)):
            emit("sp", lambda e, w_sb_=w_sb_, w_d_=w_d_, s=s: e.dma_start(
                out=w_sb_[:], in_=w_d_[:]).then_inc(s, 16))
        wp_ap = wp_d[:].rearrange("(a p) c -> p a c", p=128)
        emit("sp", lambda e, wp_ap=wp_ap: e.dma_start(
            out=wp_sb[:], in_=wp_ap).then_inc(sem_wp, 16))

        def x_slice(tx, cp, t0, n):
            """xT hi/lo slice [128, 2, n]; chunk 0 lives in the boot packs
            (hi in boot, lo in boot2 so the hi terms start one DMA earlier)."""
            if t0 + n <= QCH:
                if tx == 0:
                    return boot_sb[:, 2 * cp:2 * cp + 2, 256 + t0:256 + t0 + n]
                return boot2_sb[:, 2 * cp:2 * cp + 2, t0:t0 + n]
            return x8_sb[:, tx, 2 * cp:2 * cp + 2, t0:t0 + n]

        def wq_slice(co, tw, cp):
            if co == 0:
                return boot_sb[:, 2 * cp:2 * cp + 2, tw * 128:tw * 128 + 128]
            return wq8_sb[:, co, tw, cp, :, :]

        def wait_x(eng, tch):
            if tch == 0:
                wait(eng, sem_boot, 16)
                wait(eng, sem_boot2, 16)
            else:
                wait(eng, sem_xch[tch], 16)

        def wait_wq(eng, co):
            wait(eng, sem_boot if co == 0 else sem_wqc[co], 16)

        all_ps = proj_ps + py_ps     # QT rotates over all 6 (py idle then)
        all_tenant = [None] * 6      # list of (eng_key, cnt) per psum slot
        proj_tenant = all_tenant     # K/V/OUT use slots 0-2
        m_tenant = [None, None]
        pbf_tenant = [None, None]

        def slot_wait(eng, tenants, slot):
            t = tenants[slot]
            if t is not None:
                for ek, ecnt in t:
                    wait(eng, ENG_SEM[ek], ecnt)

        state = {"g": 0, "gp": 0}
        qt_drain = {}                # (co, tch) -> pool lo cnt
        k_drain, v_drain = {}, {}
        m_drain = {}                 # gm -> act cnt
        p_drain = {}                 # (b, hp) -> pool lo cnt

        # ---- unit emitters --------------------------------------------
        def qt_group(tch, co):
            t0 = tch * QCH
            slot = state["g"] % 6
            pq = all_ps[slot]
            wait_wq("pe", co)
            if tch == 0:
                wait("pe", sem_boot, 16)
            else:
                wait_x("pe", tch)
            slot_wait("pe", all_tenant, slot)
            idx = 0
            for tx, tw in TERMS:
                for cp in range(CP):
                    if idx == 6 and tch == 0:
                        # x_lo (boot2) only needed from the 3rd term on
                        wait("pe", sem_boot2, 16)
                    mm = lambda e, tx=tx, tw=tw, cp=cp, co=co, t0=t0, pq=pq, i=idx: e.matmul(
                        pq[:], wq_slice(co, tw, cp), x_slice(tx, cp, t0, QCH),
                        start=(i == 0), stop=(i == 8), perf_mode=DR)
                    if idx == 8:
                        emit("pe", lambda e, mm=mm: mm(e).then_inc(sem_pe))
                        cnt["pe"] += 1
                    else:
                        emit("pe", mm)
                    idx += 1
            # qt8 lives at 32x (psum scale): hi = Q8(psum + 32bq) on ACT;
            # lo = (psum + 32bq) - hi on DVE straight from PSUM. The extra
            # 32 folds into the OUT drain scale.
            wait("act", sem_b, 16)
            wait("act", sem_pe, cnt["pe"])
            emit("act", lambda e, co=co, t0=t0, pq=pq: e.activation(
                out=qt8_sb[:, 0, co, t0:t0 + QCH], in_=pq[:], func=Ident,
                bias=bq_sb[:, co:co + 1]
            ).then_inc(sem_act))
            cnt["act"] += 1
            hi_cnt = cnt["act"]
            wait("dve", sem_b, 32)
            wait("dve", sem_pe, cnt["pe"])
            wait("dve", sem_act, hi_cnt)
            emit("dve", lambda e, pq=pq, co=co, t0=t0: e.scalar_tensor_tensor(
                qt8_sb[:, 1, co, t0:t0 + QCH], pq[:], bqf_sb[:, co:co + 1],
                qt8_sb[:, 0, co, t0:t0 + QCH], ADD, SUB).then_inc(sem_dve))
            cnt["dve"] += 1
            qt_drain[(co, tch)] = cnt["dve"]
            all_tenant[slot] = [("dve", cnt["dve"])]
            state["g"] += 1

        def nat_group(tt, och, w_sb_, w_sem, dst_sb, bias_bc, drain_dict,
                      nslots=3):
            o0 = och * OCH
            slot = state["g"] % nslots
            pv = all_ps[slot]
            wait("pe", w_sem, 16)
            wait_x("pe", (tt * 128) // QCH)
            slot_wait("pe", all_tenant, slot)
            idx = 0
            for tx, tw in TERMS:
                for cp in range(CP):
                    mm = lambda e, tx=tx, tw=tw, cp=cp, tt=tt, o0=o0, pv=pv, w_sb_=w_sb_, i=idx: e.matmul(
                        pv[:, 0:OCH], x_slice(tx, cp, tt * 128, 128),
                        w_sb_[:, tw, cp, :, o0:o0 + OCH],
                        start=(i == 0), stop=(i == 8), perf_mode=DR)
                    if idx == 8:
                        emit("pe", lambda e, mm=mm: mm(e).then_inc(sem_pe))
                        cnt["pe"] += 1
                    else:
                        emit("pe", mm)
                    idx += 1
            wait("dve", sem_br, 16)
            wait("dve", sem_pe, cnt["pe"])
            emit("dve", lambda e, tt=tt, o0=o0, pv=pv, dst_sb=dst_sb, bias_bc=bias_bc:
                 e.tensor_add(dst_sb[:, tt, o0:o0 + OCH], pv[:, 0:OCH],
                              bias_bc[:, o0:o0 + OCH]).then_inc(sem_dve))
            cnt["dve"] += 1
            drain_dict[(tt, och)] = cnt["dve"]
            all_tenant[slot] = [("dve", cnt["dve"])]
            state["g"] += 1

        def m_group(b, hp):
            """MT[b,hpair] = Vh^T @ Kh (transposed M: j on partitions)."""
            gm = b * HPAIRS + hp
            slot = gm % 2
            pm = m_ps[slot]
            ochn = (hp * 128) // OCH
            ochn2 = (hp * 128 + 127) // OCH
            slot_wait("pe", m_tenant, slot)
            for kt in range(8):
                tt = b * 8 + kt
                c0 = hp * 128
                need = max(k_drain[(tt, ochn)], v_drain[(tt, ochn)],
                           k_drain[(tt, ochn2)], v_drain[(tt, ochn2)])
                wait("pe", sem_dve, need)
                emit("pe", lambda e, tt=tt, c0=c0, pm=pm, kt=kt: e.matmul(
                    pm[0:D, :], v_sb[:, tt, c0:c0 + D], k_sb[:, tt, c0:c0 + D],
                    start=(kt == 0), stop=(kt == 7), tile_position=(0, 0)))
                mm = lambda e, tt=tt, c0=c0, pm=pm, kt=kt: e.matmul(
                    pm[D:2 * D, :], v_sb[:, tt, c0 + D:c0 + 2 * D],
                    k_sb[:, tt, c0 + D:c0 + 2 * D],
                    start=(kt == 0), stop=(kt == 7), tile_position=(0, 64))
                if kt == 7:
                    emit("pe", lambda e, mm=mm: mm(e).then_inc(sem_pe))
                    cnt["pe"] += 1
                else:
                    emit("pe", mm)
            # drain diagonal blocks into the zeroed blockdiag tile
            wait("act", sem_pe, cnt["pe"])
            emit("act", lambda e, gm=gm, pm=pm: e.activation(
                out=m_sb[0:D, gm, 0:D], in_=pm[0:D, :], func=Ident,
                scale=MS).then_inc(sem_act))
            cnt["act"] += 1
            emit("act", lambda e, gm=gm, pm=pm: e.activation(
                out=m_sb[D:2 * D, gm, D:2 * D], in_=pm[D:2 * D, :], func=Ident,
                scale=MS).then_inc(sem_act))
            cnt["act"] += 1
            m_drain[gm] = cnt["act"]
            m_tenant[slot] = [("act", cnt["act"])]

        def p_group(b, hp):
            """P_pair = MT_blockdiag^T-mm vs Wp rows; hi/lo fp8 via staging.
            Two independent 384-wide halves (one PSUM bank each)."""
            gm = b * HPAIRS + hp
            wait("pe", sem_wp, 16)
            wait("pe", sem_act, m_drain[gm])
            pslot = gm % 2
            slot_wait("act", pbf_tenant, pslot)
            for och in range(2):
                o0 = och * OCH
                slot = 3 + state["gp"] % 3
                pp = all_ps[slot]
                slot_wait("pe", all_tenant, slot)
                emit("pe", lambda e, gm=gm, hp=hp, o0=o0, pp=pp: e.matmul(
                    pp[:, 0:OCH], m_sb[:, gm, :], wp_sb[:, hp, o0:o0 + OCH],
                    start=True, stop=True).then_inc(sem_pe))
                cnt["pe"] += 1
                # ACT: x4 -> bf16 staging, then hi fp8; Pool: lo = staged - hi
                wait("act", sem_pe, cnt["pe"])
                emit("act", lambda e, pp=pp, pslot=pslot, o0=o0: e.activation(
                    out=pbf_sb[:, pslot, o0:o0 + OCH], in_=pp[:, 0:OCH],
                    func=Ident, scale=4.0).then_inc(sem_act))
                cnt["act"] += 1
                all_tenant[slot] = [("act", cnt["act"])]
                emit("act", lambda e, b=b, hp=hp, pslot=pslot, o0=o0: e.copy(
                    p8_sb[:, 0, b, hp, o0:o0 + OCH],
                    pbf_sb[:, pslot, o0:o0 + OCH]).then_inc(sem_act))
                cnt["act"] += 1
                wait("pool", sem_act, cnt["act"])
                emit("pool", lambda e, b=b, hp=hp, pslot=pslot, o0=o0: e.tensor_sub(
                    p8_sb[:, 1, b, hp, o0:o0 + OCH],
                    pbf_sb[:, pslot, o0:o0 + OCH],
                    p8_sb[:, 0, b, hp, o0:o0 + OCH]).then_inc(sem_pool))
                cnt["pool"] += 1
                state["gp"] += 1
            p_drain[(b, hp)] = cnt["pool"]
            pbf_tenant[pslot] = [("pool", cnt["pool"])]

        slot_dmas = [0] * NOT

        def out_group(tt, och):
            b = tt // 8
            slot = tt % NOT
            o0 = och * OCH
            pslot = state["g"] % 3
            pz = proj_ps[pslot]
            wait("pe", sem_pool, max(p_drain[(b, hp2)] for hp2 in range(CT)))
            wait("pe", sem_dve, max(qt_drain[(co, 2 * b + (tt % 8) // 4)]
                                    for co in range(CT)))
            slot_wait("pe", proj_tenant, pslot)
            idx = 0
            for tq, tp in TERMS:
                for cp in range(CP):
                    mm = lambda e, tq=tq, tp=tp, cp=cp, tt=tt, b=b, o0=o0, pz=pz, i=idx: e.matmul(
                        pz[:, 0:OCH],
                        qt8_sb[:, tq, 2 * cp:2 * cp + 2, tt * 128:(tt + 1) * 128],
                        p8_sb[:, tp, b, 2 * cp:2 * cp + 2, o0:o0 + OCH],
                        start=(i == 0), stop=(i == 8), perf_mode=DR)
                    if idx == 8:
                        emit("pe", lambda e, mm=mm: mm(e).then_inc(sem_pe))
                        cnt["pe"] += 1
                    else:
                        emit("pe", mm)
                    idx += 1
            wait("dve", sem_br, 16)
            wait("dve", sem_pe, cnt["pe"])
            if och == 0 and tt >= NOT:
                wait("dve", sem_out[slot], 16 * slot_dmas[slot])
            # psum = 32q * 4*SCALE*M*WpT = 128*(y WpT); ot = 32*out
            last = tt == TT - 1
            emit("dve", lambda e, slot=slot, o0=o0, pz=pz: e.scalar_tensor_tensor(
                ot_sb[slot][:, o0:o0 + OCH], pz[:, 0:OCH], 0.25,
                bp_bc[:, o0:o0 + OCH], MULT, ADD).then_inc(sem_dve))
            cnt["dve"] += 1
            proj_tenant[pslot] = [("dve", cnt["dve"])]
            state["g"] += 1
            if last:
                # per-och DMA: the och0 transfer hides under och1's compute
                wait("sp", sem_dve, cnt["dve"])
                emit("sp", lambda e, tt=tt, slot=slot, o0=o0: e.dma_start(
                    out=out_d[tt * 128:(tt + 1) * 128, o0:o0 + OCH],
                    in_=ot_sb[slot][:, o0:o0 + OCH]
                ).then_inc(sem_out[slot], 16))
                slot_dmas[slot] += 1
            elif och == 1:
                wait("sp", sem_dve, cnt["dve"])
                emit("sp", lambda e, tt=tt, slot=slot: e.dma_start(
                    out=out_d[tt * 128:(tt + 1) * 128, :], in_=ot_sb[slot][:]
                ).then_inc(sem_out[slot], 16))
                slot_dmas[slot] += 1

        # ---- schedule -------------------------------------------------
        def interleave(la, lb, frac=1.0):
            """Merge work lists; lb paced to finish when la is at `frac`."""
            out, ia, ib = [], 0, 0
            while ia < len(la) or ib < len(lb):
                if ib < len(lb) and (ia >= len(la) or
                                     ib * frac * len(la) <= ia * len(lb)):
                    out.append(lb[ib]); ib += 1
                else:
                    out.append(la[ia]); ia += 1
            return out

        units = []
        for tch in range(TOK // QCH):
            for co in range(CT):
                units.append(lambda tch=tch, co=co: qt_group(tch, co))
                if tch == 3 and co == 2:
                    # hoist the K-phase weight wait into the QT stream (the
                    # DMA landed long ago): the PE.SEQ consumes it while the
                    # engine is still busy, instead of exposing it as a gap
                    # at the phase transition.
                    units.append(lambda: wait("pe", sem_wk, 16))
        # K(b0), V(b0)
        for w_sb_, w_sem, dst, bbc, dd in ((wk8_sb, sem_wk, k_sb, bk_bc, k_drain),
                                           (wv8_sb, sem_wv, v_sb, bv_bc, v_drain)):
            for tt in range(8):
                for och in range(2):
                    units.append(lambda tt=tt, och=och, w_sb_=w_sb_, w_sem=w_sem,
                                 dst=dst, bbc=bbc, dd=dd:
                                 nat_group(tt, och, w_sb_, w_sem, dst, bbc, dd,
                                           nslots=6))
        # Master window: K/V(b1) och-major (och0 for all tt first, so MT(b1)
        # pairs hp0-2 unblock halfway through) followed by OUT(b0); ALL 24
        # MT/P units paced across it so their drains spread over ~34us where
        # ACT/DVE/Pool each stay under ~75% busy.
        kv_b1 = []
        for och in range(2):
            for w_sb_, w_sem, dst, bbc, dd in ((wk8_sb, sem_wk, k_sb, bk_bc, k_drain),
                                               (wv8_sb, sem_wv, v_sb, bv_bc, v_drain)):
                for tt in range(8, 16):
                    kv_b1.append(lambda tt=tt, och=och, w_sb_=w_sb_, w_sem=w_sem,
                                 dst=dst, bbc=bbc, dd=dd:
                                 nat_group(tt, och, w_sb_, w_sem, dst, bbc, dd))

        def mp_units(b):
            """MT/P units software-pipelined: P(hp) follows MT(hp+1), hiding
            the MT ACT-drain round trip behind another PE unit."""
            ms = [lambda hp=hp, b=b: m_group(b, hp) for hp in range(HPAIRS)]
            pse = [lambda hp=hp, b=b: p_group(b, hp) for hp in range(HPAIRS)]
            out = [ms[0]]
            for i in range(1, HPAIRS):
                out += [ms[i], pse[i - 1]]
            out.append(pse[HPAIRS - 1])
            return out

        out_b0 = [lambda tt=tt, och=och: out_group(tt, och)
                  for tt in range(8) for och in range(2)]
        mk = lambda b, hp: (lambda: m_group(b, hp))
        pk = lambda b, hp: (lambda: p_group(b, hp))
        # mpA needs at most K/V(b1) och0 (done at 50% of kv_b1): MT/P(b0)
        # fully + MT(b1, hp0-2). mpB needs och1: the rest, paced over OUT(b0).
        mpA = mp_units(0) + [mk(1, 0), mk(1, 1), pk(1, 0), mk(1, 2), pk(1, 1)]
        mpB = [pk(1, 2), mk(1, 3), mk(1, 4), pk(1, 3), mk(1, 5), pk(1, 4), pk(1, 5)]
        units += interleave(kv_b1, mpA, frac=0.95)
        units += interleave(out_b0, mpB, frac=0.75)
        # tail: OUT(b1)
        units += [lambda tt=tt, och=och: out_group(tt, och)
                  for tt in range(8, 16) for och in range(2)]

        for u in units:
            u()

        for s_i in range(NOT):
            wait("sp", sem_out[s_i], 16 * slot_dmas[s_i])

        # ---------------- emit ----------------
        with nc.Block(no_gpsimd_drain=True) as block:

            @block.sync
            def _(e):
                for fn in ops["sp"]:
                    fn(e)

            @block.tensor
            def _(e):
                for fn in ops["pe"]:
                    fn(e)

            @block.scalar
            def _(e):
                for fn in ops["act"]:
                    fn(e)

            @block.vector
            def _(e):
                for fn in ops["dve"]:
                    fn(e)

            @block.gpsimd
            def _(e):
                for fn in ops["pool"]:
                    fn(e)

    return nc


def _get_nc():
    if "nc" not in _CACHE:
        _CACHE["nc"] = _build_nc()
    return _CACHE["nc"]


def _split8(a):
    hi = a.astype(float8_e4m3)
    lo = (a - hi.astype(np.float32)).astype(float8_e4m3)
    return hi, lo


def _pack_w_nat(wT32):
    """[C, C] -> hi/lo packed [128, 2, CP, 2, C]."""
    hi, lo = _split8(wT32)
    def pack(w):
        return w.reshape(CP, 2, 128, C).transpose(2, 0, 1, 3)
    return np.ascontiguousarray(np.stack([pack(hi), pack(lo)], axis=1))


def _make_in_maps(x, Wq, bq, Wk, bk, Wv, bv, Wp, bp):
    wq32 = np.ascontiguousarray(Wq.T).astype(np.float32) * WS
    wk32 = np.ascontiguousarray(Wk.T).astype(np.float32) * WS
    wv32 = np.ascontiguousarray(Wv.T).astype(np.float32) * WS

    wq_hi, wq_lo = _split8(wq32)
    # wq8[p, co, t, cp, i, j] = wq_t[(2cp+i)*128+p, co*128+j]
    def packq(w):
        return w.reshape(CP, 2, 128, CT, 128).transpose(2, 3, 0, 1, 4)
    wq8 = np.ascontiguousarray(np.stack([packq(wq_hi), packq(wq_lo)], axis=2))

    wk8 = _pack_w_nat(wk32)
    wv8 = _pack_w_nat(wv32)
    wpT = np.ascontiguousarray(Wp.T).astype(bfloat16)

    # [128, CT] partition-major at 32x: partition p, col ci = 32*bq[ci*128+p]
    bqs = (bq * WS).astype(np.float32).reshape(CT, 128).T
    bqf = np.ascontiguousarray(bqs, dtype=np.float32)
    bqb = np.ascontiguousarray(bqs.astype(bfloat16))
    brows = np.empty((128, 3 * C), dtype=bfloat16)
    brows[:, 0:C] = np.broadcast_to((bk * WS).astype(bfloat16), (128, C))
    brows[:, C:2 * C] = np.broadcast_to((bv * WS).astype(bfloat16), (128, C))
    brows[:, 2 * C:3 * C] = np.broadcast_to((bp * WS).astype(bfloat16), (128, C))

    # boot[p, ci, 0:128]=wq_hi, [128:256]=wq_lo (col-block 0)
    wq_part_hi = wq_hi[:, 0:128].reshape(CT, 128, 128).transpose(1, 0, 2)
    wq_part_lo = wq_lo[:, 0:128].reshape(CT, 128, 128).transpose(1, 0, 2)

    in_maps = []
    for c in range(N_CORES):
        xs = x[c * BP:(c + 1) * BP].reshape(TOK, C)
        xT = np.ascontiguousarray(xs.T).astype(np.float32)
        xhi, xlo = _split8(xT)
        x8T = np.ascontiguousarray(np.stack([xhi, xlo], axis=0))
        boot = np.empty((128, CT, 256 + QCH), dtype=float8_e4m3)
        boot[:, :, 0:128] = wq_part_hi
        boot[:, :, 128:256] = wq_part_lo
        boot[:, :, 256:] = xhi[:, 0:QCH].reshape(CT, 128, QCH).transpose(1, 0, 2)
        boot2 = np.ascontiguousarray(
            xlo[:, 0:QCH].reshape(CT, 128, QCH).transpose(1, 0, 2))
        in_maps.append({
            "boot": boot, "boot2": boot2, "x8T": x8T, "wq8": wq8, "wk8": wk8,
            "wv8": wv8, "wpT": wpT, "bqb": bqb, "bqf": bqf, "brows": brows,
        })
    return in_maps


def run(trace=False, tmpdir=None, **inputs):
    from concourse.bass_utils import run_bass_kernel_spmd

    inputs = {k: np.asarray(v, dtype=np.float32) for k, v in inputs.items()}
    nc = _get_nc()
    in_maps = _make_in_maps(**inputs)
    res = run_bass_kernel_spmd(nc, in_maps, core_ids=list(range(N_CORES)),
                               trace=trace, tmpdir=tmpdir)
    out = np.concatenate(
        [(res.results[c]["out"].astype(np.float32) / WS).reshape(BP, T, C)
         for c in range(N_CORES)], axis=0
    )
    return out, res


def kernel(**inputs):
    out, _ = run(trace=False, **inputs)
    return out
